# revision 21
# baseline (speedup 1.0000x reference)
"""Trainium2 Bass kernel for the DiffKS pipeline:
  x = invert_lpc(y, A_exc)         (order-6 time-varying FIR)
  out = sample_wise_lpc(x, A_loop) (order-2 time-varying all-pole IIR)

Sharding: pure data-parallel over batch B=48 -> 6 rows per core x 8 cores.

Per-core algorithm (all fp32, VectorE-centric):
  * Time axis (T=88200, padded to 128*690) is chunked across the 128 SBUF
    partitions; every chunk re-runs the recurrence from 64 samples early
    ("warmup") with zero initial state. |A_loop| <= 0.25 makes the
    homogeneous dynamics contract by >=2x per 2 samples, so the wrong
    boundary state is attenuated below ~2^-32 by the time the chunk's
    real samples start - well below fp32 noise.
  * The FIR is 12 tensor_tensor passes (6 mult + 6 add) over the chunked
    layout.
  * The order-2 IIR is solved by pair-condensation into coupled order-1
    recurrences over (even,odd) sample pairs and Gauss-Seidel sweeps where
    each half-sweep is an exact first-order solve via the hardware
    tensor_tensor_scan. Worst-case contraction per sweep is
    (0.25/(1-0.25)) * (0.0625/(1-0.3125)) ~ 0.03, so NSWEEP=3 sweeps +
    a final even half-sweep leave iteration error below fp32 rounding
    noise (measured 3.2e-7 relative vs the sequential reference).
"""

import numpy as np

import sys

for _p in ("/opt/trn_rl_repo",):
    if _p not in sys.path:
        sys.path.insert(0, _p)

from concourse import bacc, bass, mybir, tile
from concourse.bass_utils import run_bass_kernel_spmd

B, T = 48, 88200
NCORES = 8
BLOC = B // NCORES       # 6 batch rows per core
K, L = 128, 690          # chunks x chunk length; K*L = 88320 >= T
W = 64                   # warmup samples per chunk (must be even)
SEG = W + L              # 754 samples per chunk-segment
HP = SEG // 2            # 377 pairs per segment
PRE = 72                 # zeros prepended to every padded row
TP = PRE + K * L + 8     # 88400 padded row length
import os

NB = int(os.environ.get("KS_NB", "1"))       # batch rows per slab
NSLAB = BLOC // NB
NSWEEP = int(os.environ.get("KS_NSWEEP", "3"))  # Gauss-Seidel sweeps (incl. sweep 0)
# NOTE: GpSimd(Pool) offload of FIR taps / E-setup was tried and is faster in
# the cost model (~110us vs ~138us), but a Pool-instruction-heavy program
# reproducibly hangs the exec unit on this HW path (NRT_EXEC_UNIT_UNRECOVERABLE)
# even though small Pool probes pass, so everything stays on VectorE.
POOL_TAPS = int(os.environ.get("KS_POOL_TAPS", "0"))  # FIR taps on GpSimd (taps 7-PT..6)
POOL_EF = int(os.environ.get("KS_POOL_EF", "0"))      # E-setup + f2 on GpSimd
POOL_U = int(os.environ.get("KS_POOL_U", "0"))        # 0: none, 1: u2 on pool, 2: u1+u2 on pool
FINAL_HALF = int(os.environ.get("KS_FINAL", "1"))     # extra even half-sweep at the end
WIDE = int(os.environ.get("KS_WIDE", "0"))            # K=64,L=1380, 2 rows packed in partitions

MULT = mybir.AluOpType.mult
ADD = mybir.AluOpType.add

_compiled = {}


def _dram_view(handle, offset, dims):
    """Raw strided view of a DRAM tensor: dims = [(stride, count), ...]."""
    return bass.AP(handle, offset, [[s, c] for (s, c) in dims])


def _build_program_wide():
    """K=64 chunks x L=1380; partitions hold (row, chunk) = 2 rows per slab.

    Same algorithm as _build_program but with 2x longer instructions (less
    per-op overhead) and half the warmup fraction. All tiles are plain 2D
    [128, n]; partition p = row*64 + chunk.
    """
    Kw, Lw = 64, 1380
    SEGw = W + Lw            # 1444
    HPw = SEGw // 2          # 722
    nc = bacc.Bacc("TRN2", target_bir_lowering=False, debug=False)

    y_d = nc.dram_tensor("y_pad", (BLOC, TP), mybir.dt.float32, kind="ExternalInput")
    a_d = nc.dram_tensor("a_tap", (6, BLOC, TP), mybir.dt.float32, kind="ExternalInput")
    b1_d = nc.dram_tensor("b1_pad", (BLOC, TP), mybir.dt.float32, kind="ExternalInput")
    b2_d = nc.dram_tensor("b2_pad", (BLOC, TP), mybir.dt.float32, kind="ExternalInput")
    out_d = nc.dram_tensor("y_out", (BLOC, Kw * Lw), mybir.dt.float32, kind="ExternalOutput")

    v = nc.vector
    f32 = mybir.dt.float32

    def pair2(ap2, sel):
        n = ap2.shape[1]
        assert n % 2 == 0
        return ap2.rearrange("p (m two) -> p two m", two=2)[:, sel, :]

    with tile.TileContext(nc) as tc:
        with tc.tile_pool(name="main", bufs=2) as pool:
            for s in range(3):
                rows = [s * 2, s * 2 + 1]

                yt = pool.tile([128, SEGw + 8], f32, name=f"yt{s}", tag="yt")
                at = [pool.tile([128, SEGw], f32, name=f"at{k}_{s}", tag=f"at{k}") for k in range(1, 7)]
                b1t = pool.tile([128, SEGw], f32, name=f"b1t{s}", tag="b1t")
                b2t = pool.tile([128, SEGw], f32, name=f"b2t{s}", tag="b2t")
                xt = pool.tile([128, SEGw], f32, name=f"xt{s}", tag="xt")
                tmp = pool.tile([128, SEGw], f32, name=f"tmp{s}", tag="tmp")
                e10 = pool.tile([128, HPw], f32, name=f"e10_{s}", tag="e10")
                e11 = pool.tile([128, HPw], f32, name=f"e11_{s}", tag="e11")
                f2 = pool.tile([128, HPw], f32, name=f"f2_{s}", tag="f2")
                u1 = pool.tile([128, HPw], f32, name=f"u1_{s}", tag="u1")
                u2 = pool.tile([128, HPw], f32, name=f"u2_{s}", tag="u2")
                s1 = pool.tile([128, HPw + 1], f32, name=f"s1_{s}", tag="s1")
                s2 = pool.tile([128, HPw + 1], f32, name=f"s2_{s}", tag="s2")
                yo = pool.tile([128, SEGw + 2], f32, name=f"yo{s}", tag="yo")

                # ---- input DMAs: one 128-partition transfer per tile
                # (DRAM side walks row x chunk x j; SBUF partition = row*64+chunk)
                r0 = rows[0]
                nc.sync.dma_start(
                    yt[:, :],
                    _dram_view(y_d, r0 * TP + 2, [(TP, 2), (Lw, Kw), (1, SEGw + 8)]),
                )
                for k in range(1, 7):
                    nc.sync.dma_start(
                        at[k - 1][:, :],
                        _dram_view(a_d, ((k - 1) * BLOC + r0) * TP + 8, [(TP, 2), (Lw, Kw), (1, SEGw)]),
                    )
                nc.sync.dma_start(
                    b1t[:, :], _dram_view(b1_d, r0 * TP + 8, [(TP, 2), (Lw, Kw), (1, SEGw)])
                )
                nc.sync.dma_start(
                    b2t[:, :], _dram_view(b2_d, r0 * TP + 8, [(TP, 2), (Lw, Kw), (1, SEGw)])
                )

                b1e, b1o = pair2(b1t[:], 0), pair2(b1t[:], 1)
                b2e, b2o = pair2(b2t[:], 0), pair2(b2t[:], 1)

                # ---- pair condensation (no x dependency) ----
                v.tensor_mul(e10[:], b1o, b2e)
                v.tensor_mul(e11[:], b1o, b1e)
                v.tensor_add(e11[:], e11[:], b2o)

                # ---- FIR ----
                v.tensor_mul(xt[:], at[0][:], yt[:, 5 : 5 + SEGw])
                v.tensor_add(xt[:], xt[:], yt[:, 6 : 6 + SEGw])
                for k in range(2, 7):
                    v.tensor_mul(tmp[:], at[k - 1][:], yt[:, 6 - k : 6 - k + SEGw])
                    v.tensor_add(xt[:], xt[:], tmp[:])

                xe, xo = pair2(xt[:], 0), pair2(xt[:], 1)
                v.tensor_mul(f2[:], b1o, xe)
                v.tensor_add(f2[:], f2[:], xo)

                v.memset(s1[:, 0:1], 0.0)
                v.memset(s2[:, 0:1], 0.0)
                v.memset(yo[:, 0:2], 0.0)

                s1d, s1s = s1[:, 1:], s1[:, 0:HPw]
                s2d, s2s = s2[:, 1:], s2[:, 0:HPw]
                yod = yo[:, 2:]
                yo_even = pair2(yod, 0)
                yo_odd = pair2(yod, 1)
                yo_odd_sh = pair2(yo[:, 0:SEGw], 1)

                def tts2(out2, d0, d1):
                    v.tensor_tensor_scan(out2, d0, d1, 0.0, MULT, ADD)

                # ---- sweep 0 ----
                tts2(s1d, b2e, xe)
                v.tensor_mul(u2[:], e10[:], s1s)
                v.tensor_add(u2[:], u2[:], f2[:])
                tts2(s2d, e11[:], u2[:])

                for sw in range(1, NSWEEP):
                    last = sw == NSWEEP - 1
                    v.tensor_mul(u1[:], b1e, s2s)
                    v.tensor_add(u1[:], u1[:], xe)
                    tts2(s1d, b2e, u1[:])
                    v.tensor_mul(u2[:], e10[:], s1s)
                    v.tensor_add(u2[:], u2[:], f2[:])
                    tts2(yo_odd if last else s2d, e11[:], u2[:])

                v.tensor_mul(u1[:], b1e, yo_odd_sh)
                v.tensor_add(u1[:], u1[:], xe)
                tts2(yo_even, b2e, u1[:])

                nc.sync.dma_start(
                    _dram_view(out_d, r0 * Kw * Lw, [(Kw * Lw, 2), (Lw, Kw), (1, Lw)]),
                    yo[:, 2 + W : 2 + W + Lw],
                )

    nc.compile()
    return nc


def _build_program():
    if WIDE:
        return _build_program_wide()
    nc = bacc.Bacc("TRN2", target_bir_lowering=False, debug=False)

    y_d = nc.dram_tensor("y_pad", (BLOC, TP), mybir.dt.float32, kind="ExternalInput")
    a_d = nc.dram_tensor("a_tap", (6, BLOC, TP), mybir.dt.float32, kind="ExternalInput")
    b1_d = nc.dram_tensor("b1_pad", (BLOC, TP), mybir.dt.float32, kind="ExternalInput")
    b2_d = nc.dram_tensor("b2_pad", (BLOC, TP), mybir.dt.float32, kind="ExternalInput")
    out_d = nc.dram_tensor("y_out", (BLOC, K * L), mybir.dt.float32, kind="ExternalOutput")

    v = nc.vector
    g = nc.gpsimd

    def pair(ap3, sel):
        # [128, NB, 2*n] -> even (sel=0) / odd (sel=1) view [128, NB, n]
        n = ap3.shape[2]
        assert n % 2 == 0
        return ap3.rearrange("p b (m two) -> p b two m", two=2)[:, :, sel, :]

    bufs = int(os.environ.get("KS_BUFS", "2"))
    with tile.TileContext(nc) as tc:
        with tc.tile_pool(name="main", bufs=bufs) as pool:
            for s in range(NSLAB):
                rows = [s * NB + i for i in range(NB)]

                yt = pool.tile([K, NB, 762], mybir.dt.float32, name=f"yt{s}", tag="yt")
                at = [
                    pool.tile([K, NB, SEG], mybir.dt.float32, name=f"at{k}_{s}", tag=f"at{k}")
                    for k in range(1, 7)
                ]
                b1t = pool.tile([K, NB, SEG], mybir.dt.float32, name=f"b1t{s}", tag="b1t")
                b2t = pool.tile([K, NB, SEG], mybir.dt.float32, name=f"b2t{s}", tag="b2t")
                xt = pool.tile([K, NB, SEG], mybir.dt.float32, name=f"xt{s}", tag="xt")
                tmp = pool.tile([K, NB, SEG], mybir.dt.float32, name=f"tmp{s}", tag="tmp")
                if POOL_TAPS > 0:
                    pp = pool.tile([K, NB, SEG], mybir.dt.float32, name=f"pp{s}", tag="pp")
                    ptmp = pool.tile([K, NB, SEG], mybir.dt.float32, name=f"ptmp{s}", tag="ptmp")
                e10 = pool.tile([K, NB, HP], mybir.dt.float32, name=f"e10_{s}", tag="e10")
                e11 = pool.tile([K, NB, HP], mybir.dt.float32, name=f"e11_{s}", tag="e11")
                f2 = pool.tile([K, NB, HP], mybir.dt.float32, name=f"f2_{s}", tag="f2")
                u1 = pool.tile([K, NB, HP], mybir.dt.float32, name=f"u1_{s}", tag="u1")
                u2 = pool.tile([K, NB, HP], mybir.dt.float32, name=f"u2_{s}", tag="u2")
                s1 = pool.tile([K, NB, HP + 1], mybir.dt.float32, name=f"s1_{s}", tag="s1")
                s2 = pool.tile([K, NB, HP + 1], mybir.dt.float32, name=f"s2_{s}", tag="s2")
                yo = pool.tile([K, NB, SEG + 2], mybir.dt.float32, name=f"yo{s}", tag="yo")

                # ---- input DMAs (chunk-strided views of the padded rows) ----
                # Pool's FIR taps (7-POOL_TAPS..6) load first so GpSimd can
                # start before VectorE finishes its own taps.
                _order = os.environ.get("KS_TAP_ORDER", "seq")
                if _order == "pool_first":
                    tap_order = list(range(7 - POOL_TAPS, 7)) + list(range(1, 7 - POOL_TAPS))
                elif _order == "interleave":
                    a_, b_ = list(range(1, 7 - POOL_TAPS)), list(range(7 - POOL_TAPS, 7))
                    tap_order = [x for pair_ in zip(a_, b_) for x in pair_]
                    tap_order += a_[len(b_):] + b_[len(a_):]
                else:
                    tap_order = list(range(1, 7))
                for i, r in enumerate(rows):
                    nc.sync.dma_start(
                        yt[:, i, :], _dram_view(y_d, r * TP + 2, [(L, K), (1, 762)])
                    )
                    for k in tap_order:
                        nc.sync.dma_start(
                            at[k - 1][:, i, :],
                            _dram_view(a_d, ((k - 1) * BLOC + r) * TP + 8, [(L, K), (1, SEG)]),
                        )
                    nc.sync.dma_start(
                        b1t[:, i, :], _dram_view(b1_d, r * TP + 8, [(L, K), (1, SEG)])
                    )
                    nc.sync.dma_start(
                        b2t[:, i, :], _dram_view(b2_d, r * TP + 8, [(L, K), (1, SEG)])
                    )

                xe, xo = pair(xt, 0), pair(xt, 1)
                b1e, b1o = pair(b1t, 0), pair(b1t, 1)
                b2e, b2o = pair(b2t, 0), pair(b2t, 1)

                # ---- pair condensation (E has no x dependency: emit first) ----
                ee = g if POOL_EF >= 1 else v
                ee.tensor_mul(e10[:], b1o, b2e)
                ee.tensor_mul(e11[:], b1o, b1e)
                ee.tensor_add(e11[:], e11[:], b2o)

                # ---- FIR: x[j] = y[j] + sum_k A_k[j] * y[j-k] ----
                # taps 1..6-POOL_TAPS accumulate on VectorE; the top POOL_TAPS
                # taps are multiplied+summed on GpSimd and added in once.
                dve_hi = 6 - POOL_TAPS
                if POOL_TAPS > 0:
                    k0 = dve_hi + 1
                    g.tensor_mul(pp[:], at[k0 - 1][:], yt[:, :, 6 - k0 : 6 - k0 + SEG])
                    for k in range(k0 + 1, 7):
                        g.tensor_mul(ptmp[:], at[k - 1][:], yt[:, :, 6 - k : 6 - k + SEG])
                        g.tensor_add(pp[:], pp[:], ptmp[:])
                v.tensor_mul(xt[:], at[0][:], yt[:, :, 5 : 5 + SEG])
                v.tensor_add(xt[:], xt[:], yt[:, :, 6 : 6 + SEG])
                for k in range(2, dve_hi + 1):
                    v.tensor_mul(tmp[:], at[k - 1][:], yt[:, :, 6 - k : 6 - k + SEG])
                    v.tensor_add(xt[:], xt[:], tmp[:])
                if POOL_TAPS > 0:
                    v.tensor_add(xt[:], xt[:], pp[:])

                # ---- f2 (needs x) ----
                fe = g if POOL_EF == 1 else v
                fe.tensor_mul(f2[:], b1o, xe)
                fe.tensor_add(f2[:], f2[:], xo)

                # guard columns (shift reads at m=0 land here; must be finite)
                v.memset(s1[:, :, 0:1], 0.0)
                v.memset(s2[:, :, 0:1], 0.0)
                v.memset(yo[:, :, 0:2], 0.0)

                s1d, s1s = s1[:, :, 1:], s1[:, :, 0:HP]
                s2d, s2s = s2[:, :, 1:], s2[:, :, 0:HP]
                yod = yo[:, :, 2:]                      # [K, NB, SEG]
                yo_even = pair(yod, 0)                  # write: y at even pairs
                yo_odd = pair(yod, 1)                   # write: y at odd pairs
                yo_odd_sh = pair(yo[:, :, 0:SEG], 1)    # read: odd pairs shifted by 1
                yo_even_sh = pair(yo[:, :, 0:SEG], 0)   # read: even pairs shifted by 1

                def tts(out3, d0_3, d1_3):
                    for i in range(NB):
                        v.tensor_tensor_scan(
                            out3[:, i, :], d0_3[:, i, :], d1_3[:, i, :], 0.0, MULT, ADD
                        )

                u2e = g if POOL_U >= 1 else v
                u1e = g if POOL_U >= 2 else v

                # ---- sweep 0 (s2_prev = 0) ----
                tts(s1d, b2e, xe)
                u2e.tensor_mul(u2[:], e10[:], s1s)
                u2e.tensor_add(u2[:], u2[:], f2[:])
                tts(s2d, e11, u2)

                # ---- sweeps 1..NSWEEP-1 ----
                for sw in range(1, NSWEEP):
                    last = sw == NSWEEP - 1
                    s1_out = yo_even if (last and not FINAL_HALF) else s1d
                    s1_sh = yo_even_sh if (last and not FINAL_HALF) else s1s
                    u1e.tensor_mul(u1[:], b1e, s2s)
                    u1e.tensor_add(u1[:], u1[:], xe)
                    tts(s1_out, b2e, u1)
                    u2e.tensor_mul(u2[:], e10[:], s1_sh)
                    u2e.tensor_add(u2[:], u2[:], f2[:])
                    tts(yo_odd if last else s2d, e11, u2)

                if FINAL_HALF:
                    # ---- final even half-sweep against the settled odd samples ----
                    u1e.tensor_mul(u1[:], b1e, yo_odd_sh)
                    u1e.tensor_add(u1[:], u1[:], xe)
                    tts(yo_even, b2e, u1)

                # ---- output DMA (drop warmup; tail pad cut on host) ----
                for i, r in enumerate(rows):
                    nc.sync.dma_start(
                        _dram_view(out_d, r * K * L, [(L, K), (1, L)]),
                        yo[:, i, 2 + W : 2 + SEG],
                    )

    nc.compile()
    return nc


def _prep_inputs(y, A_exc, A_loop):
    y = np.ascontiguousarray(y, dtype=np.float32)
    A_exc = np.ascontiguousarray(A_exc, dtype=np.float32)
    A_loop = np.ascontiguousarray(A_loop, dtype=np.float32)

    y_pad = np.zeros((B, TP), np.float32)
    y_pad[:, PRE : PRE + T] = y
    a_tap = np.zeros((6, B, TP), np.float32)
    for k in range(6):
        a_tap[k, :, PRE : PRE + T] = A_exc[:, :, k]
    b1_pad = np.zeros((B, TP), np.float32)
    b2_pad = np.zeros((B, TP), np.float32)
    b1_pad[:, PRE : PRE + T] = -A_loop[:, :, 0]
    b2_pad[:, PRE : PRE + T] = -A_loop[:, :, 1]

    in_maps = []
    for c in range(NCORES):
        r0, r1 = c * BLOC, (c + 1) * BLOC
        in_maps.append(
            {
                "y_pad": y_pad[r0:r1],
                "a_tap": np.ascontiguousarray(a_tap[:, r0:r1]),
                "b1_pad": b1_pad[r0:r1],
                "b2_pad": b2_pad[r0:r1],
            }
        )
    return in_maps


def _get_program():
    if "nc" not in _compiled:
        _compiled["nc"] = _build_program()
    return _compiled["nc"]


def run(y, A_exc, A_loop, trace=False, **trace_kwargs):
    """Returns (output, BassKernelResults)."""
    nc = _get_program()
    in_maps = _prep_inputs(y, A_exc, A_loop)
    res = run_bass_kernel_spmd(
        nc, in_maps, list(range(NCORES)), trace=trace, **trace_kwargs
    )
    out = np.empty((B, T), np.float32)
    for c in range(NCORES):
        out[c * BLOC : (c + 1) * BLOC] = res.results[c]["y_out"][:, :T]
    return out, res


def kernel(y, A_exc, A_loop):
    out, _ = run(y, A_exc, A_loop)
    return out


# revision 25
# speedup vs baseline: 1.1217x; 1.1217x over previous
"""Trainium2 Bass kernel for the DiffKS pipeline:
  x = invert_lpc(y, A_exc)         (order-6 time-varying FIR)
  out = sample_wise_lpc(x, A_loop) (order-2 time-varying all-pole IIR)

Sharding: pure data-parallel over batch B=48 -> 6 rows per core x 8 cores.

Per-core algorithm (all fp32, VectorE-centric):
  * Time axis (T=88200, padded to 128*690) is chunked across the 128 SBUF
    partitions; every chunk re-runs the recurrence from 64 samples early
    ("warmup") with zero initial state. |A_loop| <= 0.25 makes the
    homogeneous dynamics contract by >=2x per 2 samples, so the wrong
    boundary state is attenuated below ~2^-32 by the time the chunk's
    real samples start - well below fp32 noise.
  * The FIR is 12 tensor_tensor passes (6 mult + 6 add) over the chunked
    layout.
  * The order-2 IIR is solved by pair-condensation into coupled order-1
    recurrences over (even,odd) sample pairs and Gauss-Seidel sweeps where
    each half-sweep is an exact first-order solve via the hardware
    tensor_tensor_scan. Worst-case contraction per sweep is
    (0.25/(1-0.25)) * (0.0625/(1-0.3125)) ~ 0.03, so NSWEEP=3 sweeps +
    a final even half-sweep leave iteration error below fp32 rounding
    noise (measured 3.2e-7 relative vs the sequential reference).
"""

import numpy as np

import sys

for _p in ("/opt/trn_rl_repo",):
    if _p not in sys.path:
        sys.path.insert(0, _p)

from concourse import bacc, bass, mybir, tile
from concourse.bass_utils import run_bass_kernel_spmd

B, T = 48, 88200
NCORES = 8
BLOC = B // NCORES       # 6 batch rows per core
K, L = 128, 690          # chunks x chunk length; K*L = 88320 >= T
W = 64                   # warmup samples per chunk (must be even)
SEG = W + L              # 754 samples per chunk-segment
HP = SEG // 2            # 377 pairs per segment
PRE = 72                 # zeros prepended to every padded row
TP = PRE + K * L + 8     # 88400 padded row length
import os

NB = int(os.environ.get("KS_NB", "1"))       # batch rows per slab
NSLAB = BLOC // NB
NSWEEP = int(os.environ.get("KS_NSWEEP", "3"))  # Gauss-Seidel sweeps (incl. sweep 0)
# NOTE on GpSimd(Pool) offload: a Pool-instruction-heavy program (40+ pool
# ops, fine-grained DVE interleave) reproducibly hangs the exec unit on this
# HW path (NRT_EXEC_UNIT_UNRECOVERABLE) even though small Pool probes pass.
# The shipping config uses the WIDE layout with only 15 coarse pool ops
# (3 FIR tap multiplies + 2 adds per slab, one cross-engine edge per slab),
# which runs clean on HW and was validated at rel err 3.1e-7.
POOL_TAPS = int(os.environ.get("KS_POOL_TAPS", "3"))  # FIR taps on GpSimd (taps 7-PT..6)
POOL_EF = int(os.environ.get("KS_POOL_EF", "0"))      # E-setup + f2 on GpSimd
POOL_U = int(os.environ.get("KS_POOL_U", "0"))        # 0: none, 1: u2 on pool, 2: u1+u2 on pool
FINAL_HALF = int(os.environ.get("KS_FINAL", "1"))     # extra even half-sweep at the end
WIDE = int(os.environ.get("KS_WIDE", "1"))            # K=64,L=1380, 2 rows packed in partitions

MULT = mybir.AluOpType.mult
ADD = mybir.AluOpType.add

_compiled = {}


def _dram_view(handle, offset, dims):
    """Raw strided view of a DRAM tensor: dims = [(stride, count), ...]."""
    return bass.AP(handle, offset, [[s, c] for (s, c) in dims])


def _build_program_wide():
    """K=64 chunks x L=1380; partitions hold (row, chunk) = 2 rows per slab.

    Same algorithm as _build_program but with 2x longer instructions (less
    per-op overhead) and half the warmup fraction. All tiles are plain 2D
    [128, n]; partition p = row*64 + chunk.
    """
    Kw, Lw = 64, 1380
    SEGw = W + Lw            # 1444
    HPw = SEGw // 2          # 722
    nc = bacc.Bacc("TRN2", target_bir_lowering=False, debug=False)

    y_d = nc.dram_tensor("y_pad", (BLOC, TP), mybir.dt.float32, kind="ExternalInput")
    a_d = nc.dram_tensor("a_tap", (6, BLOC, TP), mybir.dt.float32, kind="ExternalInput")
    b1_d = nc.dram_tensor("b1_pad", (BLOC, TP), mybir.dt.float32, kind="ExternalInput")
    b2_d = nc.dram_tensor("b2_pad", (BLOC, TP), mybir.dt.float32, kind="ExternalInput")
    out_d = nc.dram_tensor("y_out", (BLOC, Kw * Lw), mybir.dt.float32, kind="ExternalOutput")

    v = nc.vector
    g = nc.gpsimd
    f32 = mybir.dt.float32

    def pair2(ap2, sel):
        n = ap2.shape[1]
        assert n % 2 == 0
        return ap2.rearrange("p (m two) -> p two m", two=2)[:, sel, :]

    with tile.TileContext(nc) as tc:
        with tc.tile_pool(name="main", bufs=2) as pool:
            for s in range(3):
                rows = [s * 2, s * 2 + 1]

                yt = pool.tile([128, SEGw + 8], f32, name=f"yt{s}", tag="yt")
                at = [pool.tile([128, SEGw], f32, name=f"at{k}_{s}", tag=f"at{k}") for k in range(1, 7)]
                b1t = pool.tile([128, SEGw], f32, name=f"b1t{s}", tag="b1t")
                b2t = pool.tile([128, SEGw], f32, name=f"b2t{s}", tag="b2t")
                xt = pool.tile([128, SEGw], f32, name=f"xt{s}", tag="xt")
                tmp = pool.tile([128, SEGw], f32, name=f"tmp{s}", tag="tmp")
                if POOL_TAPS > 0:
                    pp = pool.tile([128, SEGw], f32, name=f"pp{s}", tag="pp")
                    ptmp = pool.tile([128, SEGw], f32, name=f"ptmp{s}", tag="ptmp")
                e10 = pool.tile([128, HPw], f32, name=f"e10_{s}", tag="e10")
                e11 = pool.tile([128, HPw], f32, name=f"e11_{s}", tag="e11")
                f2 = pool.tile([128, HPw], f32, name=f"f2_{s}", tag="f2")
                u1 = pool.tile([128, HPw], f32, name=f"u1_{s}", tag="u1")
                u2 = pool.tile([128, HPw], f32, name=f"u2_{s}", tag="u2")
                s1 = pool.tile([128, HPw + 1], f32, name=f"s1_{s}", tag="s1")
                s2 = pool.tile([128, HPw + 1], f32, name=f"s2_{s}", tag="s2")
                yo = pool.tile([128, SEGw + 2], f32, name=f"yo{s}", tag="yo")

                # ---- input DMAs: one 128-partition transfer per tile
                # (DRAM side walks row x chunk x j; SBUF partition = row*64+chunk)
                r0 = rows[0]
                nc.sync.dma_start(
                    yt[:, :],
                    _dram_view(y_d, r0 * TP + 2, [(TP, 2), (Lw, Kw), (1, SEGw + 8)]),
                )
                for k in range(1, 7):
                    nc.sync.dma_start(
                        at[k - 1][:, :],
                        _dram_view(a_d, ((k - 1) * BLOC + r0) * TP + 8, [(TP, 2), (Lw, Kw), (1, SEGw)]),
                    )
                nc.sync.dma_start(
                    b1t[:, :], _dram_view(b1_d, r0 * TP + 8, [(TP, 2), (Lw, Kw), (1, SEGw)])
                )
                nc.sync.dma_start(
                    b2t[:, :], _dram_view(b2_d, r0 * TP + 8, [(TP, 2), (Lw, Kw), (1, SEGw)])
                )

                b1e, b1o = pair2(b1t[:], 0), pair2(b1t[:], 1)
                b2e, b2o = pair2(b2t[:], 0), pair2(b2t[:], 1)

                # ---- pair condensation (no x dependency) ----
                v.tensor_mul(e10[:], b1o, b2e)
                v.tensor_mul(e11[:], b1o, b1e)
                v.tensor_add(e11[:], e11[:], b2o)

                # ---- FIR (top POOL_TAPS taps multiplied+summed on GpSimd) ----
                dve_hi = 6 - POOL_TAPS
                if POOL_TAPS > 0:
                    k0 = dve_hi + 1
                    g.tensor_mul(pp[:], at[k0 - 1][:], yt[:, 6 - k0 : 6 - k0 + SEGw])
                    for k in range(k0 + 1, 7):
                        g.tensor_mul(ptmp[:], at[k - 1][:], yt[:, 6 - k : 6 - k + SEGw])
                        g.tensor_add(pp[:], pp[:], ptmp[:])
                v.tensor_mul(xt[:], at[0][:], yt[:, 5 : 5 + SEGw])
                v.tensor_add(xt[:], xt[:], yt[:, 6 : 6 + SEGw])
                for k in range(2, dve_hi + 1):
                    v.tensor_mul(tmp[:], at[k - 1][:], yt[:, 6 - k : 6 - k + SEGw])
                    v.tensor_add(xt[:], xt[:], tmp[:])
                if POOL_TAPS > 0:
                    v.tensor_add(xt[:], xt[:], pp[:])

                xe, xo = pair2(xt[:], 0), pair2(xt[:], 1)
                v.tensor_mul(f2[:], b1o, xe)
                v.tensor_add(f2[:], f2[:], xo)

                v.memset(s1[:, 0:1], 0.0)
                v.memset(s2[:, 0:1], 0.0)
                v.memset(yo[:, 0:2], 0.0)

                s1d, s1s = s1[:, 1:], s1[:, 0:HPw]
                s2d, s2s = s2[:, 1:], s2[:, 0:HPw]
                yod = yo[:, 2:]
                yo_even = pair2(yod, 0)
                yo_odd = pair2(yod, 1)
                yo_odd_sh = pair2(yo[:, 0:SEGw], 1)

                def tts2(out2, d0, d1):
                    v.tensor_tensor_scan(out2, d0, d1, 0.0, MULT, ADD)

                # ---- sweep 0 ----
                tts2(s1d, b2e, xe)
                v.tensor_mul(u2[:], e10[:], s1s)
                v.tensor_add(u2[:], u2[:], f2[:])
                tts2(s2d, e11[:], u2[:])

                for sw in range(1, NSWEEP):
                    last = sw == NSWEEP - 1
                    v.tensor_mul(u1[:], b1e, s2s)
                    v.tensor_add(u1[:], u1[:], xe)
                    tts2(s1d, b2e, u1[:])
                    v.tensor_mul(u2[:], e10[:], s1s)
                    v.tensor_add(u2[:], u2[:], f2[:])
                    tts2(yo_odd if last else s2d, e11[:], u2[:])

                v.tensor_mul(u1[:], b1e, yo_odd_sh)
                v.tensor_add(u1[:], u1[:], xe)
                tts2(yo_even, b2e, u1[:])

                nc.sync.dma_start(
                    _dram_view(out_d, r0 * Kw * Lw, [(Kw * Lw, 2), (Lw, Kw), (1, Lw)]),
                    yo[:, 2 + W : 2 + W + Lw],
                )

    nc.compile()
    return nc


def _build_program():
    if WIDE:
        return _build_program_wide()
    nc = bacc.Bacc("TRN2", target_bir_lowering=False, debug=False)

    y_d = nc.dram_tensor("y_pad", (BLOC, TP), mybir.dt.float32, kind="ExternalInput")
    a_d = nc.dram_tensor("a_tap", (6, BLOC, TP), mybir.dt.float32, kind="ExternalInput")
    b1_d = nc.dram_tensor("b1_pad", (BLOC, TP), mybir.dt.float32, kind="ExternalInput")
    b2_d = nc.dram_tensor("b2_pad", (BLOC, TP), mybir.dt.float32, kind="ExternalInput")
    out_d = nc.dram_tensor("y_out", (BLOC, K * L), mybir.dt.float32, kind="ExternalOutput")

    v = nc.vector
    g = nc.gpsimd

    def pair(ap3, sel):
        # [128, NB, 2*n] -> even (sel=0) / odd (sel=1) view [128, NB, n]
        n = ap3.shape[2]
        assert n % 2 == 0
        return ap3.rearrange("p b (m two) -> p b two m", two=2)[:, :, sel, :]

    bufs = int(os.environ.get("KS_BUFS", "2"))
    with tile.TileContext(nc) as tc:
        with tc.tile_pool(name="main", bufs=bufs) as pool:
            for s in range(NSLAB):
                rows = [s * NB + i for i in range(NB)]

                yt = pool.tile([K, NB, 762], mybir.dt.float32, name=f"yt{s}", tag="yt")
                at = [
                    pool.tile([K, NB, SEG], mybir.dt.float32, name=f"at{k}_{s}", tag=f"at{k}")
                    for k in range(1, 7)
                ]
                b1t = pool.tile([K, NB, SEG], mybir.dt.float32, name=f"b1t{s}", tag="b1t")
                b2t = pool.tile([K, NB, SEG], mybir.dt.float32, name=f"b2t{s}", tag="b2t")
                xt = pool.tile([K, NB, SEG], mybir.dt.float32, name=f"xt{s}", tag="xt")
                tmp = pool.tile([K, NB, SEG], mybir.dt.float32, name=f"tmp{s}", tag="tmp")
                if POOL_TAPS > 0:
                    pp = pool.tile([K, NB, SEG], mybir.dt.float32, name=f"pp{s}", tag="pp")
                    ptmp = pool.tile([K, NB, SEG], mybir.dt.float32, name=f"ptmp{s}", tag="ptmp")
                e10 = pool.tile([K, NB, HP], mybir.dt.float32, name=f"e10_{s}", tag="e10")
                e11 = pool.tile([K, NB, HP], mybir.dt.float32, name=f"e11_{s}", tag="e11")
                f2 = pool.tile([K, NB, HP], mybir.dt.float32, name=f"f2_{s}", tag="f2")
                u1 = pool.tile([K, NB, HP], mybir.dt.float32, name=f"u1_{s}", tag="u1")
                u2 = pool.tile([K, NB, HP], mybir.dt.float32, name=f"u2_{s}", tag="u2")
                s1 = pool.tile([K, NB, HP + 1], mybir.dt.float32, name=f"s1_{s}", tag="s1")
                s2 = pool.tile([K, NB, HP + 1], mybir.dt.float32, name=f"s2_{s}", tag="s2")
                yo = pool.tile([K, NB, SEG + 2], mybir.dt.float32, name=f"yo{s}", tag="yo")

                # ---- input DMAs (chunk-strided views of the padded rows) ----
                # Pool's FIR taps (7-POOL_TAPS..6) load first so GpSimd can
                # start before VectorE finishes its own taps.
                _order = os.environ.get("KS_TAP_ORDER", "seq")
                if _order == "pool_first":
                    tap_order = list(range(7 - POOL_TAPS, 7)) + list(range(1, 7 - POOL_TAPS))
                elif _order == "interleave":
                    a_, b_ = list(range(1, 7 - POOL_TAPS)), list(range(7 - POOL_TAPS, 7))
                    tap_order = [x for pair_ in zip(a_, b_) for x in pair_]
                    tap_order += a_[len(b_):] + b_[len(a_):]
                else:
                    tap_order = list(range(1, 7))
                for i, r in enumerate(rows):
                    nc.sync.dma_start(
                        yt[:, i, :], _dram_view(y_d, r * TP + 2, [(L, K), (1, 762)])
                    )
                    for k in tap_order:
                        nc.sync.dma_start(
                            at[k - 1][:, i, :],
                            _dram_view(a_d, ((k - 1) * BLOC + r) * TP + 8, [(L, K), (1, SEG)]),
                        )
                    nc.sync.dma_start(
                        b1t[:, i, :], _dram_view(b1_d, r * TP + 8, [(L, K), (1, SEG)])
                    )
                    nc.sync.dma_start(
                        b2t[:, i, :], _dram_view(b2_d, r * TP + 8, [(L, K), (1, SEG)])
                    )

                xe, xo = pair(xt, 0), pair(xt, 1)
                b1e, b1o = pair(b1t, 0), pair(b1t, 1)
                b2e, b2o = pair(b2t, 0), pair(b2t, 1)

                # ---- pair condensation (E has no x dependency: emit first) ----
                ee = g if POOL_EF >= 1 else v
                ee.tensor_mul(e10[:], b1o, b2e)
                ee.tensor_mul(e11[:], b1o, b1e)
                ee.tensor_add(e11[:], e11[:], b2o)

                # ---- FIR: x[j] = y[j] + sum_k A_k[j] * y[j-k] ----
                # taps 1..6-POOL_TAPS accumulate on VectorE; the top POOL_TAPS
                # taps are multiplied+summed on GpSimd and added in once.
                dve_hi = 6 - POOL_TAPS
                if POOL_TAPS > 0:
                    k0 = dve_hi + 1
                    g.tensor_mul(pp[:], at[k0 - 1][:], yt[:, :, 6 - k0 : 6 - k0 + SEG])
                    for k in range(k0 + 1, 7):
                        g.tensor_mul(ptmp[:], at[k - 1][:], yt[:, :, 6 - k : 6 - k + SEG])
                        g.tensor_add(pp[:], pp[:], ptmp[:])
                v.tensor_mul(xt[:], at[0][:], yt[:, :, 5 : 5 + SEG])
                v.tensor_add(xt[:], xt[:], yt[:, :, 6 : 6 + SEG])
                for k in range(2, dve_hi + 1):
                    v.tensor_mul(tmp[:], at[k - 1][:], yt[:, :, 6 - k : 6 - k + SEG])
                    v.tensor_add(xt[:], xt[:], tmp[:])
                if POOL_TAPS > 0:
                    v.tensor_add(xt[:], xt[:], pp[:])

                # ---- f2 (needs x) ----
                fe = g if POOL_EF == 1 else v
                fe.tensor_mul(f2[:], b1o, xe)
                fe.tensor_add(f2[:], f2[:], xo)

                # guard columns (shift reads at m=0 land here; must be finite)
                v.memset(s1[:, :, 0:1], 0.0)
                v.memset(s2[:, :, 0:1], 0.0)
                v.memset(yo[:, :, 0:2], 0.0)

                s1d, s1s = s1[:, :, 1:], s1[:, :, 0:HP]
                s2d, s2s = s2[:, :, 1:], s2[:, :, 0:HP]
                yod = yo[:, :, 2:]                      # [K, NB, SEG]
                yo_even = pair(yod, 0)                  # write: y at even pairs
                yo_odd = pair(yod, 1)                   # write: y at odd pairs
                yo_odd_sh = pair(yo[:, :, 0:SEG], 1)    # read: odd pairs shifted by 1
                yo_even_sh = pair(yo[:, :, 0:SEG], 0)   # read: even pairs shifted by 1

                def tts(out3, d0_3, d1_3):
                    for i in range(NB):
                        v.tensor_tensor_scan(
                            out3[:, i, :], d0_3[:, i, :], d1_3[:, i, :], 0.0, MULT, ADD
                        )

                u2e = g if POOL_U >= 1 else v
                u1e = g if POOL_U >= 2 else v

                # ---- sweep 0 (s2_prev = 0) ----
                tts(s1d, b2e, xe)
                u2e.tensor_mul(u2[:], e10[:], s1s)
                u2e.tensor_add(u2[:], u2[:], f2[:])
                tts(s2d, e11, u2)

                # ---- sweeps 1..NSWEEP-1 ----
                for sw in range(1, NSWEEP):
                    last = sw == NSWEEP - 1
                    s1_out = yo_even if (last and not FINAL_HALF) else s1d
                    s1_sh = yo_even_sh if (last and not FINAL_HALF) else s1s
                    u1e.tensor_mul(u1[:], b1e, s2s)
                    u1e.tensor_add(u1[:], u1[:], xe)
                    tts(s1_out, b2e, u1)
                    u2e.tensor_mul(u2[:], e10[:], s1_sh)
                    u2e.tensor_add(u2[:], u2[:], f2[:])
                    tts(yo_odd if last else s2d, e11, u2)

                if FINAL_HALF:
                    # ---- final even half-sweep against the settled odd samples ----
                    u1e.tensor_mul(u1[:], b1e, yo_odd_sh)
                    u1e.tensor_add(u1[:], u1[:], xe)
                    tts(yo_even, b2e, u1)

                # ---- output DMA (drop warmup; tail pad cut on host) ----
                for i, r in enumerate(rows):
                    nc.sync.dma_start(
                        _dram_view(out_d, r * K * L, [(L, K), (1, L)]),
                        yo[:, i, 2 + W : 2 + SEG],
                    )

    nc.compile()
    return nc


def _prep_inputs(y, A_exc, A_loop):
    y = np.ascontiguousarray(y, dtype=np.float32)
    A_exc = np.ascontiguousarray(A_exc, dtype=np.float32)
    A_loop = np.ascontiguousarray(A_loop, dtype=np.float32)

    y_pad = np.zeros((B, TP), np.float32)
    y_pad[:, PRE : PRE + T] = y
    a_tap = np.zeros((6, B, TP), np.float32)
    for k in range(6):
        a_tap[k, :, PRE : PRE + T] = A_exc[:, :, k]
    b1_pad = np.zeros((B, TP), np.float32)
    b2_pad = np.zeros((B, TP), np.float32)
    b1_pad[:, PRE : PRE + T] = -A_loop[:, :, 0]
    b2_pad[:, PRE : PRE + T] = -A_loop[:, :, 1]

    in_maps = []
    for c in range(NCORES):
        r0, r1 = c * BLOC, (c + 1) * BLOC
        in_maps.append(
            {
                "y_pad": y_pad[r0:r1],
                "a_tap": np.ascontiguousarray(a_tap[:, r0:r1]),
                "b1_pad": b1_pad[r0:r1],
                "b2_pad": b2_pad[r0:r1],
            }
        )
    return in_maps


def _get_program():
    if "nc" not in _compiled:
        _compiled["nc"] = _build_program()
    return _compiled["nc"]


def run(y, A_exc, A_loop, trace=False, **trace_kwargs):
    """Returns (output, BassKernelResults)."""
    nc = _get_program()
    in_maps = _prep_inputs(y, A_exc, A_loop)
    res = run_bass_kernel_spmd(
        nc, in_maps, list(range(NCORES)), trace=trace, **trace_kwargs
    )
    out = np.empty((B, T), np.float32)
    for c in range(NCORES):
        out[c * BLOC : (c + 1) * BLOC] = res.results[c]["y_out"][:, :T]
    return out, res


def kernel(y, A_exc, A_loop):
    out, _ = run(y, A_exc, A_loop)
    return out


# revision 28
# speedup vs baseline: 1.1897x; 1.0607x over previous
"""Trainium2 Bass kernel for the DiffKS pipeline:
  x = invert_lpc(y, A_exc)         (order-6 time-varying FIR)
  out = sample_wise_lpc(x, A_loop) (order-2 time-varying all-pole IIR)

Sharding: pure data-parallel over batch B=48 -> 6 rows per core x 8 cores.

Per-core algorithm (all fp32, VectorE-centric):
  * Time axis (T=88200, padded to 128*690) is chunked across the 128 SBUF
    partitions; every chunk re-runs the recurrence from 64 samples early
    ("warmup") with zero initial state. |A_loop| <= 0.25 makes the
    homogeneous dynamics contract by >=2x per 2 samples, so the wrong
    boundary state is attenuated below ~2^-32 by the time the chunk's
    real samples start - well below fp32 noise.
  * The FIR is 12 tensor_tensor passes (6 mult + 6 add) over the chunked
    layout.
  * The order-2 IIR is solved by pair-condensation into coupled order-1
    recurrences over (even,odd) sample pairs and Gauss-Seidel sweeps where
    each half-sweep is an exact first-order solve via the hardware
    tensor_tensor_scan. Worst-case contraction per sweep is
    (0.25/(1-0.25)) * (0.0625/(1-0.3125)) ~ 0.03, so NSWEEP=3 sweeps +
    a final even half-sweep leave iteration error below fp32 rounding
    noise (measured 3.2e-7 relative vs the sequential reference).
"""

import numpy as np

import sys

for _p in ("/opt/trn_rl_repo",):
    if _p not in sys.path:
        sys.path.insert(0, _p)

from concourse import bacc, bass, mybir, tile
from concourse.bass_utils import run_bass_kernel_spmd

B, T = 48, 88200
NCORES = 8
BLOC = B // NCORES       # 6 batch rows per core
K, L = 128, 690          # chunks x chunk length; K*L = 88320 >= T
W = 64                   # warmup samples per chunk (must be even)
SEG = W + L              # 754 samples per chunk-segment
HP = SEG // 2            # 377 pairs per segment
PRE = 72                 # zeros prepended to every padded row
TP = PRE + K * L + 8     # 88400 padded row length
import os

NB = int(os.environ.get("KS_NB", "1"))       # batch rows per slab
NSLAB = BLOC // NB
NSWEEP = int(os.environ.get("KS_NSWEEP", "3"))  # Gauss-Seidel sweeps (incl. sweep 0)
# NOTE on GpSimd(Pool) offload: a Pool-instruction-heavy program (40+ pool
# ops, fine-grained DVE interleave) reproducibly hangs the exec unit on this
# HW path (NRT_EXEC_UNIT_UNRECOVERABLE) even though small Pool probes pass.
# The shipping config uses the WIDE layout with only 15 coarse pool ops
# (3 FIR tap multiplies + 2 adds per slab, one cross-engine edge per slab),
# which runs clean on HW and was validated at rel err 3.1e-7.
POOL_TAPS = int(os.environ.get("KS_POOL_TAPS", "3"))  # FIR taps on GpSimd (taps 7-PT..6)
POOL_EF = int(os.environ.get("KS_POOL_EF", "0"))      # E-setup + f2 on GpSimd
POOL_U = int(os.environ.get("KS_POOL_U", "0"))        # 0: none, 1: u2 on pool, 2: u1+u2 on pool
FINAL_HALF = int(os.environ.get("KS_FINAL", "1"))     # extra even half-sweep at the end
WIDE = int(os.environ.get("KS_WIDE", "1"))            # K=64,L=1380, 2 rows packed in partitions
POOL_LOW = int(os.environ.get("KS_POOL_LOW", "0"))    # pool owns taps 1..PT (DMAs land first)

MULT = mybir.AluOpType.mult
ADD = mybir.AluOpType.add

_compiled = {}


def _dram_view(handle, offset, dims):
    """Raw strided view of a DRAM tensor: dims = [(stride, count), ...]."""
    return bass.AP(handle, offset, [[s, c] for (s, c) in dims])


def _build_program_wide():
    """K=64 chunks x L=1380; partitions hold (row, chunk) = 2 rows per slab.

    Same algorithm as _build_program but with 2x longer instructions (less
    per-op overhead) and half the warmup fraction. All tiles are plain 2D
    [128, n]; partition p = row*64 + chunk.
    """
    Kw, Lw = 64, 1380
    SEGw = W + Lw            # 1444
    HPw = SEGw // 2          # 722
    nc = bacc.Bacc("TRN2", target_bir_lowering=False, debug=False)

    y_d = nc.dram_tensor("y_pad", (BLOC, TP), mybir.dt.float32, kind="ExternalInput")
    a_d = nc.dram_tensor("a_tap", (6, BLOC, TP), mybir.dt.float32, kind="ExternalInput")
    b1_d = nc.dram_tensor("b1_pad", (BLOC, TP), mybir.dt.float32, kind="ExternalInput")
    b2_d = nc.dram_tensor("b2_pad", (BLOC, TP), mybir.dt.float32, kind="ExternalInput")
    out_d = nc.dram_tensor("y_out", (BLOC, Kw * Lw), mybir.dt.float32, kind="ExternalOutput")

    v = nc.vector
    g = nc.gpsimd
    f32 = mybir.dt.float32

    def pair2(ap2, sel):
        n = ap2.shape[1]
        assert n % 2 == 0
        return ap2.rearrange("p (m two) -> p two m", two=2)[:, sel, :]

    with tile.TileContext(nc) as tc:
        with tc.tile_pool(name="main", bufs=2) as pool:
            for s in range(3):
                rows = [s * 2, s * 2 + 1]

                yt = pool.tile([128, SEGw + 8], f32, name=f"yt{s}", tag="yt")
                at = [pool.tile([128, SEGw], f32, name=f"at{k}_{s}", tag=f"at{k}") for k in range(1, 7)]
                b1t = pool.tile([128, SEGw], f32, name=f"b1t{s}", tag="b1t")
                b2t = pool.tile([128, SEGw], f32, name=f"b2t{s}", tag="b2t")
                xt = pool.tile([128, SEGw], f32, name=f"xt{s}", tag="xt")
                tmp = pool.tile([128, SEGw], f32, name=f"tmp{s}", tag="tmp")
                if POOL_TAPS > 0:
                    pp = pool.tile([128, SEGw], f32, name=f"pp{s}", tag="pp")
                    ptmp = pool.tile([128, SEGw], f32, name=f"ptmp{s}", tag="ptmp")
                e10 = pool.tile([128, HPw], f32, name=f"e10_{s}", tag="e10")
                e11 = pool.tile([128, HPw], f32, name=f"e11_{s}", tag="e11")
                f2 = pool.tile([128, HPw], f32, name=f"f2_{s}", tag="f2")
                u1 = pool.tile([128, HPw], f32, name=f"u1_{s}", tag="u1")
                u2 = pool.tile([128, HPw], f32, name=f"u2_{s}", tag="u2")
                s1 = pool.tile([128, HPw + 1], f32, name=f"s1_{s}", tag="s1")
                s2 = pool.tile([128, HPw + 1], f32, name=f"s2_{s}", tag="s2")
                yo = pool.tile([128, SEGw + 2], f32, name=f"yo{s}", tag="yo")

                # ---- input DMAs: one 128-partition transfer per tile
                # (DRAM side walks row x chunk x j; SBUF partition = row*64+chunk)
                # Order: yt, then the pool-owned taps, then b1/b2 (DVE E-setup),
                # then the DVE-owned taps - so both engines start ASAP.
                r0 = rows[0]
                pool_taps = list(range(1, POOL_TAPS + 1)) if POOL_LOW else list(range(7 - POOL_TAPS, 7))
                dve_taps = [k for k in range(1, 7) if k not in pool_taps]
                nc.sync.dma_start(
                    yt[:, :],
                    _dram_view(y_d, r0 * TP + 2, [(TP, 2), (Lw, Kw), (1, SEGw + 8)]),
                )
                for k in pool_taps + [0, -1] + dve_taps:
                    if k == 0:
                        nc.sync.dma_start(
                            b1t[:, :], _dram_view(b1_d, r0 * TP + 8, [(TP, 2), (Lw, Kw), (1, SEGw)])
                        )
                    elif k == -1:
                        nc.sync.dma_start(
                            b2t[:, :], _dram_view(b2_d, r0 * TP + 8, [(TP, 2), (Lw, Kw), (1, SEGw)])
                        )
                    else:
                        nc.sync.dma_start(
                            at[k - 1][:, :],
                            _dram_view(a_d, ((k - 1) * BLOC + r0) * TP + 8, [(TP, 2), (Lw, Kw), (1, SEGw)]),
                        )

                b1e, b1o = pair2(b1t[:], 0), pair2(b1t[:], 1)
                b2e, b2o = pair2(b2t[:], 0), pair2(b2t[:], 1)

                # ---- pair condensation (no x dependency) ----
                v.tensor_mul(e10[:], b1o, b2e)
                v.tensor_mul(e11[:], b1o, b1e)
                v.tensor_add(e11[:], e11[:], b2o)

                # ---- FIR (POOL_TAPS taps multiplied+summed on GpSimd) ----
                if POOL_TAPS > 0:
                    pk = pool_taps
                    g.tensor_mul(pp[:], at[pk[0] - 1][:], yt[:, 6 - pk[0] : 6 - pk[0] + SEGw])
                    for k in pk[1:]:
                        g.tensor_mul(ptmp[:], at[k - 1][:], yt[:, 6 - k : 6 - k + SEGw])
                        g.tensor_add(pp[:], pp[:], ptmp[:])
                dk = dve_taps
                v.tensor_mul(xt[:], at[dk[0] - 1][:], yt[:, 6 - dk[0] : 6 - dk[0] + SEGw])
                v.tensor_add(xt[:], xt[:], yt[:, 6 : 6 + SEGw])
                for k in dk[1:]:
                    v.tensor_mul(tmp[:], at[k - 1][:], yt[:, 6 - k : 6 - k + SEGw])
                    v.tensor_add(xt[:], xt[:], tmp[:])
                if POOL_TAPS > 0:
                    v.tensor_add(xt[:], xt[:], pp[:])

                xe, xo = pair2(xt[:], 0), pair2(xt[:], 1)
                v.tensor_mul(f2[:], b1o, xe)
                v.tensor_add(f2[:], f2[:], xo)

                v.memset(s1[:, 0:1], 0.0)
                v.memset(s2[:, 0:1], 0.0)
                v.memset(yo[:, 0:2], 0.0)

                s1d, s1s = s1[:, 1:], s1[:, 0:HPw]
                s2d, s2s = s2[:, 1:], s2[:, 0:HPw]
                yod = yo[:, 2:]
                yo_even = pair2(yod, 0)
                yo_odd = pair2(yod, 1)
                yo_odd_sh = pair2(yo[:, 0:SEGw], 1)

                def tts2(out2, d0, d1):
                    v.tensor_tensor_scan(out2, d0, d1, 0.0, MULT, ADD)

                # ---- sweep 0 ----
                tts2(s1d, b2e, xe)
                v.tensor_mul(u2[:], e10[:], s1s)
                v.tensor_add(u2[:], u2[:], f2[:])
                tts2(s2d, e11[:], u2[:])

                for sw in range(1, NSWEEP):
                    last = sw == NSWEEP - 1
                    v.tensor_mul(u1[:], b1e, s2s)
                    v.tensor_add(u1[:], u1[:], xe)
                    tts2(s1d, b2e, u1[:])
                    v.tensor_mul(u2[:], e10[:], s1s)
                    v.tensor_add(u2[:], u2[:], f2[:])
                    tts2(yo_odd if last else s2d, e11[:], u2[:])

                v.tensor_mul(u1[:], b1e, yo_odd_sh)
                v.tensor_add(u1[:], u1[:], xe)
                tts2(yo_even, b2e, u1[:])

                nc.sync.dma_start(
                    _dram_view(out_d, r0 * Kw * Lw, [(Kw * Lw, 2), (Lw, Kw), (1, Lw)]),
                    yo[:, 2 + W : 2 + W + Lw],
                )

    nc.compile()
    return nc


def _build_program():
    if WIDE:
        return _build_program_wide()
    nc = bacc.Bacc("TRN2", target_bir_lowering=False, debug=False)

    y_d = nc.dram_tensor("y_pad", (BLOC, TP), mybir.dt.float32, kind="ExternalInput")
    a_d = nc.dram_tensor("a_tap", (6, BLOC, TP), mybir.dt.float32, kind="ExternalInput")
    b1_d = nc.dram_tensor("b1_pad", (BLOC, TP), mybir.dt.float32, kind="ExternalInput")
    b2_d = nc.dram_tensor("b2_pad", (BLOC, TP), mybir.dt.float32, kind="ExternalInput")
    out_d = nc.dram_tensor("y_out", (BLOC, K * L), mybir.dt.float32, kind="ExternalOutput")

    v = nc.vector
    g = nc.gpsimd

    def pair(ap3, sel):
        # [128, NB, 2*n] -> even (sel=0) / odd (sel=1) view [128, NB, n]
        n = ap3.shape[2]
        assert n % 2 == 0
        return ap3.rearrange("p b (m two) -> p b two m", two=2)[:, :, sel, :]

    bufs = int(os.environ.get("KS_BUFS", "2"))
    with tile.TileContext(nc) as tc:
        with tc.tile_pool(name="main", bufs=bufs) as pool:
            for s in range(NSLAB):
                rows = [s * NB + i for i in range(NB)]

                yt = pool.tile([K, NB, 762], mybir.dt.float32, name=f"yt{s}", tag="yt")
                at = [
                    pool.tile([K, NB, SEG], mybir.dt.float32, name=f"at{k}_{s}", tag=f"at{k}")
                    for k in range(1, 7)
                ]
                b1t = pool.tile([K, NB, SEG], mybir.dt.float32, name=f"b1t{s}", tag="b1t")
                b2t = pool.tile([K, NB, SEG], mybir.dt.float32, name=f"b2t{s}", tag="b2t")
                xt = pool.tile([K, NB, SEG], mybir.dt.float32, name=f"xt{s}", tag="xt")
                tmp = pool.tile([K, NB, SEG], mybir.dt.float32, name=f"tmp{s}", tag="tmp")
                if POOL_TAPS > 0:
                    pp = pool.tile([K, NB, SEG], mybir.dt.float32, name=f"pp{s}", tag="pp")
                    ptmp = pool.tile([K, NB, SEG], mybir.dt.float32, name=f"ptmp{s}", tag="ptmp")
                e10 = pool.tile([K, NB, HP], mybir.dt.float32, name=f"e10_{s}", tag="e10")
                e11 = pool.tile([K, NB, HP], mybir.dt.float32, name=f"e11_{s}", tag="e11")
                f2 = pool.tile([K, NB, HP], mybir.dt.float32, name=f"f2_{s}", tag="f2")
                u1 = pool.tile([K, NB, HP], mybir.dt.float32, name=f"u1_{s}", tag="u1")
                u2 = pool.tile([K, NB, HP], mybir.dt.float32, name=f"u2_{s}", tag="u2")
                s1 = pool.tile([K, NB, HP + 1], mybir.dt.float32, name=f"s1_{s}", tag="s1")
                s2 = pool.tile([K, NB, HP + 1], mybir.dt.float32, name=f"s2_{s}", tag="s2")
                yo = pool.tile([K, NB, SEG + 2], mybir.dt.float32, name=f"yo{s}", tag="yo")

                # ---- input DMAs (chunk-strided views of the padded rows) ----
                # Pool's FIR taps (7-POOL_TAPS..6) load first so GpSimd can
                # start before VectorE finishes its own taps.
                _order = os.environ.get("KS_TAP_ORDER", "seq")
                if _order == "pool_first":
                    tap_order = list(range(7 - POOL_TAPS, 7)) + list(range(1, 7 - POOL_TAPS))
                elif _order == "interleave":
                    a_, b_ = list(range(1, 7 - POOL_TAPS)), list(range(7 - POOL_TAPS, 7))
                    tap_order = [x for pair_ in zip(a_, b_) for x in pair_]
                    tap_order += a_[len(b_):] + b_[len(a_):]
                else:
                    tap_order = list(range(1, 7))
                for i, r in enumerate(rows):
                    nc.sync.dma_start(
                        yt[:, i, :], _dram_view(y_d, r * TP + 2, [(L, K), (1, 762)])
                    )
                    for k in tap_order:
                        nc.sync.dma_start(
                            at[k - 1][:, i, :],
                            _dram_view(a_d, ((k - 1) * BLOC + r) * TP + 8, [(L, K), (1, SEG)]),
                        )
                    nc.sync.dma_start(
                        b1t[:, i, :], _dram_view(b1_d, r * TP + 8, [(L, K), (1, SEG)])
                    )
                    nc.sync.dma_start(
                        b2t[:, i, :], _dram_view(b2_d, r * TP + 8, [(L, K), (1, SEG)])
                    )

                xe, xo = pair(xt, 0), pair(xt, 1)
                b1e, b1o = pair(b1t, 0), pair(b1t, 1)
                b2e, b2o = pair(b2t, 0), pair(b2t, 1)

                # ---- pair condensation (E has no x dependency: emit first) ----
                ee = g if POOL_EF >= 1 else v
                ee.tensor_mul(e10[:], b1o, b2e)
                ee.tensor_mul(e11[:], b1o, b1e)
                ee.tensor_add(e11[:], e11[:], b2o)

                # ---- FIR: x[j] = y[j] + sum_k A_k[j] * y[j-k] ----
                # taps 1..6-POOL_TAPS accumulate on VectorE; the top POOL_TAPS
                # taps are multiplied+summed on GpSimd and added in once.
                dve_hi = 6 - POOL_TAPS
                if POOL_TAPS > 0:
                    k0 = dve_hi + 1
                    g.tensor_mul(pp[:], at[k0 - 1][:], yt[:, :, 6 - k0 : 6 - k0 + SEG])
                    for k in range(k0 + 1, 7):
                        g.tensor_mul(ptmp[:], at[k - 1][:], yt[:, :, 6 - k : 6 - k + SEG])
                        g.tensor_add(pp[:], pp[:], ptmp[:])
                v.tensor_mul(xt[:], at[0][:], yt[:, :, 5 : 5 + SEG])
                v.tensor_add(xt[:], xt[:], yt[:, :, 6 : 6 + SEG])
                for k in range(2, dve_hi + 1):
                    v.tensor_mul(tmp[:], at[k - 1][:], yt[:, :, 6 - k : 6 - k + SEG])
                    v.tensor_add(xt[:], xt[:], tmp[:])
                if POOL_TAPS > 0:
                    v.tensor_add(xt[:], xt[:], pp[:])

                # ---- f2 (needs x) ----
                fe = g if POOL_EF == 1 else v
                fe.tensor_mul(f2[:], b1o, xe)
                fe.tensor_add(f2[:], f2[:], xo)

                # guard columns (shift reads at m=0 land here; must be finite)
                v.memset(s1[:, :, 0:1], 0.0)
                v.memset(s2[:, :, 0:1], 0.0)
                v.memset(yo[:, :, 0:2], 0.0)

                s1d, s1s = s1[:, :, 1:], s1[:, :, 0:HP]
                s2d, s2s = s2[:, :, 1:], s2[:, :, 0:HP]
                yod = yo[:, :, 2:]                      # [K, NB, SEG]
                yo_even = pair(yod, 0)                  # write: y at even pairs
                yo_odd = pair(yod, 1)                   # write: y at odd pairs
                yo_odd_sh = pair(yo[:, :, 0:SEG], 1)    # read: odd pairs shifted by 1
                yo_even_sh = pair(yo[:, :, 0:SEG], 0)   # read: even pairs shifted by 1

                def tts(out3, d0_3, d1_3):
                    for i in range(NB):
                        v.tensor_tensor_scan(
                            out3[:, i, :], d0_3[:, i, :], d1_3[:, i, :], 0.0, MULT, ADD
                        )

                u2e = g if POOL_U >= 1 else v
                u1e = g if POOL_U >= 2 else v

                # ---- sweep 0 (s2_prev = 0) ----
                tts(s1d, b2e, xe)
                u2e.tensor_mul(u2[:], e10[:], s1s)
                u2e.tensor_add(u2[:], u2[:], f2[:])
                tts(s2d, e11, u2)

                # ---- sweeps 1..NSWEEP-1 ----
                for sw in range(1, NSWEEP):
                    last = sw == NSWEEP - 1
                    s1_out = yo_even if (last and not FINAL_HALF) else s1d
                    s1_sh = yo_even_sh if (last and not FINAL_HALF) else s1s
                    u1e.tensor_mul(u1[:], b1e, s2s)
                    u1e.tensor_add(u1[:], u1[:], xe)
                    tts(s1_out, b2e, u1)
                    u2e.tensor_mul(u2[:], e10[:], s1_sh)
                    u2e.tensor_add(u2[:], u2[:], f2[:])
                    tts(yo_odd if last else s2d, e11, u2)

                if FINAL_HALF:
                    # ---- final even half-sweep against the settled odd samples ----
                    u1e.tensor_mul(u1[:], b1e, yo_odd_sh)
                    u1e.tensor_add(u1[:], u1[:], xe)
                    tts(yo_even, b2e, u1)

                # ---- output DMA (drop warmup; tail pad cut on host) ----
                for i, r in enumerate(rows):
                    nc.sync.dma_start(
                        _dram_view(out_d, r * K * L, [(L, K), (1, L)]),
                        yo[:, i, 2 + W : 2 + SEG],
                    )

    nc.compile()
    return nc


def _prep_inputs(y, A_exc, A_loop):
    y = np.ascontiguousarray(y, dtype=np.float32)
    A_exc = np.ascontiguousarray(A_exc, dtype=np.float32)
    A_loop = np.ascontiguousarray(A_loop, dtype=np.float32)

    y_pad = np.zeros((B, TP), np.float32)
    y_pad[:, PRE : PRE + T] = y
    a_tap = np.zeros((6, B, TP), np.float32)
    for k in range(6):
        a_tap[k, :, PRE : PRE + T] = A_exc[:, :, k]
    b1_pad = np.zeros((B, TP), np.float32)
    b2_pad = np.zeros((B, TP), np.float32)
    b1_pad[:, PRE : PRE + T] = -A_loop[:, :, 0]
    b2_pad[:, PRE : PRE + T] = -A_loop[:, :, 1]

    in_maps = []
    for c in range(NCORES):
        r0, r1 = c * BLOC, (c + 1) * BLOC
        in_maps.append(
            {
                "y_pad": y_pad[r0:r1],
                "a_tap": np.ascontiguousarray(a_tap[:, r0:r1]),
                "b1_pad": b1_pad[r0:r1],
                "b2_pad": b2_pad[r0:r1],
            }
        )
    return in_maps


def _get_program():
    if "nc" not in _compiled:
        _compiled["nc"] = _build_program()
    return _compiled["nc"]


def run(y, A_exc, A_loop, trace=False, **trace_kwargs):
    """Returns (output, BassKernelResults)."""
    nc = _get_program()
    in_maps = _prep_inputs(y, A_exc, A_loop)
    res = run_bass_kernel_spmd(
        nc, in_maps, list(range(NCORES)), trace=trace, **trace_kwargs
    )
    out = np.empty((B, T), np.float32)
    for c in range(NCORES):
        out[c * BLOC : (c + 1) * BLOC] = res.results[c]["y_out"][:, :T]
    return out, res


def kernel(y, A_exc, A_loop):
    out, _ = run(y, A_exc, A_loop)
    return out
